# revision 26
# baseline (speedup 1.0000x reference)
"""Bass/Trainium2 kernel for nn_BlastocystAuxLoss.

Computes a masked MSE over B=16,777,216 elements:
    late stages are labels 8..15; target[s] = (s-8) * 4/7 for late stages;
    loss = sum_{s>=8} (x - target)^2 / count(s>=8)   (0.0 if count == 0)

Strategy: trivially data-parallel over 8 NeuronCores. Each core reads its
B/8 shard of blast_scores (f32) and stage_labels (i32) from HBM, computes
per-partition partial {count, sse} on-chip (DVE + ACT engines, bf16
elementwise math, f32 accumulation), and writes a [128, 2] partial-sums
tile. The final scalar reduction (8*128 partials -> sse/cnt) happens on
host in f64. No collectives needed.

Per-element identities used (s = label, x = score):
    mask  m = (s >= 8)
    target t = relu(s * 4/7 - 32/7)        (== (s-8)*4/7 clamped at 0)
    sse  += (m * (bf16(x) - t))^2          (m^2 == m)
    cnt  += m
"""

from contextlib import ExitStack

import numpy as np

B = 16777216
N_CORES = 8
SHARD = B // N_CORES  # 2,097,152
P = 128

_NC_CACHE = {}


def build(shard=SHARD, n_tiles=8):
    """Build the single-core Bass program (same SPMD program for all cores)."""
    import concourse.bacc as bacc
    import concourse.tile as tile
    from concourse import mybir

    free = shard // P
    fd = free // n_tiles
    assert fd * n_tiles * P == shard

    nc = bacc.Bacc("TRN2", target_bir_lowering=False)
    x_ext = nc.declare_dram_parameter(
        "blast_scores", [shard], mybir.dt.float32, isOutput=False
    )
    s_ext = nc.declare_dram_parameter(
        "stage_labels", [shard], mybir.dt.int32, isOutput=False
    )
    out_ext = nc.declare_dram_parameter("out", [P, 2], mybir.dt.float32, isOutput=True)

    x_v = x_ext.ap().rearrange("(p f) -> p f", p=P)
    s_v = s_ext.ap().rearrange("(p f) -> p f", p=P)

    c47 = 4.0 / 7.0  # target step; folded into the Square's input scale
    c74 = 7.0 / 4.0  # x prescale so z = 7/4*(x - t) uses integer-exact v

    f32 = mybir.dt.float32
    bf16 = mybir.dt.bfloat16
    Alu = mybir.AluOpType
    Act = mybir.ActivationFunctionType

    with tile.TileContext(nc) as tc:
        with (
            tc.tile_pool(name="io", bufs=4) as io_pool,
            tc.tile_pool(name="mid", bufs=3) as mid_pool,
            tc.tile_pool(name="acc", bufs=1) as acc_pool,
        ):
            cnt_acc = acc_pool.tile([P, n_tiles], f32)
            sse_acc = acc_pool.tile([P, n_tiles], f32)
            red = acc_pool.tile([P, 2], f32)
            # bias for the sigmoid step mask: m = sigmoid(64*s - 480)
            sig_bias = acc_pool.tile([P, 1], f32)
            nc.gpsimd.memset(sig_bias[:], -480.0)

            for k in range(n_tiles):
                x_t = io_pool.tile([P, fd], f32, tag="x")
                s_t = io_pool.tile([P, fd], mybir.dt.int32, tag="s")
                nc.sync.dma_start(out=x_t[:], in_=x_v[:, k * fd : (k + 1) * fd])
                nc.sync.dma_start(out=s_t[:], in_=s_v[:, k * fd : (k + 1) * fd])

                m = mid_pool.tile([P, fd], bf16, tag="m")
                v = mid_pool.tile([P, fd], bf16, tag="v")
                z = mid_pool.tile([P, fd], bf16, tag="z")
                zm = mid_pool.tile([P, fd], bf16, tag="zm")
                sq = mid_pool.tile([P, fd], bf16, tag="sq")

                # ACT: step mask m = sigmoid(64*(s - 7.5)) in {0,1} exactly
                # (saturated at +-32); accumulate count for free
                nc.scalar.activation(
                    m[:], s_t[:], Act.Sigmoid, bias=sig_bias[:], scale=64.0,
                    accum_out=cnt_acc[:, k : k + 1],
                )
                # DVE: v = max(s-8, 0)
                nc.vector.tensor_scalar(v[:], s_t[:], 8, 0, Alu.subtract, Alu.max)
                # DVE: z = 7/4*x - v  (== 7/4*(x - target) since v = 7/4*t)
                nc.vector.scalar_tensor_tensor(
                    z[:], x_t[:], c74, v[:], Alu.mult, Alu.subtract
                )
                nc.vector.tensor_tensor(zm[:], z[:], m[:], Alu.mult)
                # ACT: sse += (4/7 * zm)^2 over masked elements
                nc.scalar.activation(
                    sq[:], zm[:], Act.Square, scale=c47,
                    accum_out=sse_acc[:, k : k + 1],
                )

            nc.vector.reduce_sum(red[:, 0:1], cnt_acc[:], axis=mybir.AxisListType.X)
            nc.vector.reduce_sum(red[:, 1:2], sse_acc[:], axis=mybir.AxisListType.X)
            nc.sync.dma_start(out=out_ext.ap()[:, :], in_=red[:])

    nc.finalize()
    return nc


def build_raw(shard=2097152, sizes=None, ring=6):
    """Hand-scheduled raw-Bass builder (no TileContext).

    - per-slot DMA semaphores (multi-queue completions are unordered);
      slot reuse (tile k vs k+R) is ordered by issue-side consumer waits
    - ring of 6 slots so DMA issue never gates on compute and the input
      stream stays bandwidth-bound end to end
    - tile sizes taper at the end so the last tile's compute lag after
      the final (bandwidth-bound) DMA is minimal
    - final reduction via a TensorEngine ones-matmul (cross-partition sum
      -> PSUM [1, 2*NT]) so the output DMA is one small descriptor instead
      of 128 8-byte ones
    """
    import concourse.bacc as bacc
    from concourse import mybir

    free = shard // P
    if sizes is None:
        sizes = [2048] * 7 + [1792, 256]
        if sum(sizes) != free:  # non-default shard (tests)
            fd = free // 8
            sizes = [fd] * 8
    assert sum(sizes) == free
    fd = max(sizes)
    NT = len(sizes)
    offs = [sum(sizes[:i]) for i in range(NT)]
    R = min(ring, NT)

    nc = bacc.Bacc("TRN2", target_bir_lowering=False)
    x_ext = nc.declare_dram_parameter(
        "blast_scores", [shard], mybir.dt.float32, isOutput=False
    )
    s_ext = nc.declare_dram_parameter(
        "stage_labels", [shard], mybir.dt.int32, isOutput=False
    )
    out_ext = nc.declare_dram_parameter("out", [2 * NT], mybir.dt.float32, isOutput=True)

    x_v = x_ext.ap().rearrange("(p f) -> p f", p=P)
    s_v = s_ext.ap().rearrange("(p f) -> p f", p=P)

    c47 = 4.0 / 7.0
    c74 = 7.0 / 4.0

    f32 = mybir.dt.float32
    i32 = mybir.dt.int32
    bf16 = mybir.dt.bfloat16
    Alu = mybir.AluOpType
    Act = mybir.ActivationFunctionType

    x_t = [nc.alloc_sbuf_tensor(f"x{i}", [P, fd], f32).ap() for i in range(R)]
    s_t = [nc.alloc_sbuf_tensor(f"s{i}", [P, fd], i32).ap() for i in range(R)]
    m_t = [nc.alloc_sbuf_tensor(f"m{i}", [P, fd], bf16).ap() for i in range(R)]
    v_t = [nc.alloc_sbuf_tensor(f"v{i}", [P, fd], bf16).ap() for i in range(2)]
    z_t = [nc.alloc_sbuf_tensor(f"z{i}", [P, fd], bf16).ap() for i in range(2)]
    zm_t = [nc.alloc_sbuf_tensor(f"zm{i}", [P, fd], bf16).ap() for i in range(R)]
    sq_t = nc.alloc_sbuf_tensor("sq", [P, fd], bf16).ap()
    # acc[:, k] = per-partition count of tile k; acc[:, NT+k] = partial sse
    acc = nc.alloc_sbuf_tensor("acc", [P, 2 * NT], f32).ap()
    red1 = nc.alloc_sbuf_tensor("red1", [1, 2 * NT], f32).ap()
    sig_bias = nc.alloc_sbuf_tensor("sig_bias", [P, 1], f32).ap()
    ones = nc.const_aps.tensor(1.0, (P, 1), f32)

    with ExitStack() as ctx:
        dma_x = [ctx.enter_context(nc.semaphore(f"dma_x{i}")) for i in range(R)]
        dma_s = [ctx.enter_context(nc.semaphore(f"dma_s{i}")) for i in range(R)]
        dve = ctx.enter_context(nc.semaphore("dve"))
        act = ctx.enter_context(nc.semaphore("act"))
        mm = ctx.enter_context(nc.semaphore("mm"))
        outd = ctx.enter_context(nc.semaphore("outd"))
        bias_rdy = ctx.enter_context(nc.semaphore("bias_rdy"))
        psum = ctx.enter_context(nc.psum_tensor("ps", [1, 2 * NT], f32))
        block = ctx.enter_context(nc.Block())

        # Semaphore increment ledger:
        #   DVE: 3 per tile (v, z, zm)            -> 3*NT total
        #   ACT: 2 per tile (m, sq) + final copy  -> 2*NT + 1 total
        #   DMA slot sems: +16 per transfer into that slot

        @block.sync
        def _(sync):
            for k in range(NT):
                i = k % R
                w = sizes[k]
                if k >= R:
                    # x slot free when z(k-R) done; s slot free when
                    # v(k-R) (implied by z) and m(k-R) done
                    sync.wait_ge(dve, 3 * (k - R) + 2)
                    sync.wait_ge(act, 2 * (k - R) + 1)
                if k >= NT - 2:
                    # tail tiles: issue late so they do not fair-share the
                    # HBM beam with the bulk stream and land right at the
                    # bandwidth-bound floor instead of smearing past it
                    sync.wait_ge(dve, 3 * (k - 3) + 2)
                sync.dma_start(
                    out=s_t[i][:, :w], in_=s_v[:, offs[k] : offs[k] + w]
                ).then_inc(dma_s[i], 16)
                sync.dma_start(
                    out=x_t[i][:, :w], in_=x_v[:, offs[k] : offs[k] + w]
                ).then_inc(dma_x[i], 16)
            sync.wait_ge(act, 2 * NT + 1)  # final ScE copy done
            sync.dma_start(out=out_ext.ap()[:], in_=red1[0:1, :]).then_inc(outd, 16)
            sync.wait_ge(outd, 16)

        @block.vector
        def _(vector):
            vector.memset(sig_bias[:, :], -480.0).then_inc(bias_rdy, 1)
            for k in range(NT):
                i = k % R
                w = sizes[k]
                rnd = 16 * (k // R + 1)
                # v = max(s-8, 0)
                vector.wait_ge(dma_s[i], rnd)
                vector.tensor_scalar(
                    v_t[k % 2][:, :w], s_t[i][:, :w], 8, 0, Alu.subtract, Alu.max
                ).then_inc(dve, 1)
                # z = 7/4*x - v
                vector.wait_ge(dma_x[i], rnd)
                vector.wait_ge(dve, 3 * k + 1)  # v(k) drained
                vector.scalar_tensor_tensor(
                    z_t[k % 2][:, :w], x_t[i][:, :w], c74, v_t[k % 2][:, :w],
                    Alu.mult, Alu.subtract,
                ).then_inc(dve, 1)
                # zm = z * m   (m(k) ready when act >= 2k+1)
                vector.wait_ge(act, 2 * k + 1)
                vector.wait_ge(dve, 3 * k + 2)  # z(k) drained
                vector.tensor_tensor(
                    zm_t[i][:, :w], z_t[k % 2][:, :w], m_t[i][:, :w], Alu.mult
                ).then_inc(dve, 1)

        @block.scalar
        def _(scalar):
            scalar.wait_ge(bias_rdy, 1)
            for k in range(NT):
                i = k % R
                w = sizes[k]
                rnd = 16 * (k // R + 1)
                # m = sigmoid(64*s - 480) in {0,1}; count accumulates free
                scalar.wait_ge(dma_s[i], rnd)
                if k >= R:
                    # m slot free when zm(k-R) done
                    scalar.wait_ge(dve, 3 * (k - R) + 3)
                scalar.activation(
                    m_t[i][:, :w], s_t[i][:, :w], Act.Sigmoid,
                    bias=sig_bias[:, :], scale=64.0,
                    accum_out=acc[:, k : k + 1],
                ).then_inc(act, 1)
                # sq = Square(zm * 4/7); sse accum; zm(k): dve >= 3k+3
                scalar.wait_ge(dve, 3 * k + 3)
                scalar.activation(
                    sq_t[:, :w], zm_t[i][:, :w], Act.Square, scale=c47,
                    accum_out=acc[:, NT + k : NT + k + 1],
                ).then_inc(act, 1)
            # after the matmul: PSUM -> SBUF single-partition copy
            scalar.wait_ge(mm, 1)
            scalar.activation(red1[0:1, :], psum.ap()[0:1, :], Act.Copy).then_inc(
                act, 1
            )

        @block.tensor
        def _(tensor):
            # cross-partition reduction: ones.T @ acc -> [1, 2*NT]
            tensor.wait_ge(act, 2 * NT)
            tensor.wait_ge(dve, 3 * NT)
            tensor.matmul(psum.ap()[0:1, :], ones, acc[:, :]).then_inc(mm, 1)

    nc.finalize()
    return nc


def run(x, s, **spmd_kwargs):
    """Shard, run on 8 cores, host-reduce. Returns (loss, BassKernelResults)."""
    from concourse.bass_utils import run_bass_kernel_spmd

    if "nc" not in _NC_CACHE:
        _NC_CACHE["nc"] = build_raw()
    nc = _NC_CACHE["nc"]

    in_maps = [
        {
            "blast_scores": x[i * SHARD : (i + 1) * SHARD],
            "stage_labels": s[i * SHARD : (i + 1) * SHARD],
        }
        for i in range(N_CORES)
    ]
    res = run_bass_kernel_spmd(nc, in_maps, core_ids=list(range(N_CORES)), **spmd_kwargs)

    cnt = 0.0
    sse = 0.0
    for r in res.results:
        o = r["out"].astype(np.float64).reshape(2, -1)
        cnt += o[0].sum()
        sse += o[1].sum()
    val = sse / max(cnt, 1.0) if cnt > 0 else 0.0
    return np.asarray(val, dtype=np.float32), res


def kernel(**inputs):
    x = np.ascontiguousarray(np.asarray(inputs["blast_scores"], dtype=np.float32))
    s = np.ascontiguousarray(np.asarray(inputs["stage_labels"], dtype=np.int32))
    assert x.shape == (B,) and s.shape == (B,)
    return run(x, s)[0]
